# revision 1
# baseline (speedup 1.0000x reference)
"""Multi-head attention (B=2, T=2048, H=8, K=128) on 8 TRN2 NeuronCores.

Sharding: tensor-parallel over heads — core c owns head c for both batches.
Each core computes its head's attention output projected through its slice
of Wu (a partial sum over the unified dim); the host sums the 8 partials
and adds the bias.

Per-core dataflow (everything "transposed": features on partitions, tokens
on the moving/free axis). All big matmuls run in bf16 with fp32 PSUM
accumulation (bf16 streams at 1 cyc/row on the PE; fp32r measures ~3.7).
Softmax statistics, normalization and the output projection stay fp32(r).

  x_bf  = bf16(x)        [t-tiles, k]    DVE cast
  X^T   [k=128, t=4096]  bf16 PE transposes of 32 [128,128] tiles
  Q^T/K^T/V^T = W^T X^T  [128, 4096]     bf16 matmuls (V,K,Q interleaved)
  V     [s-chunks, j]    bf16 PE transposes of V^T
  per 1024-token block (software-pipelined over 128-key chunks s):
      S^T_s = K_s Q^T              [128, 1024] PSUM fp32
      E_s   = exp(S^T_s/sqrt(128)) ACT -> SBUF bf16
      sumexp += ones^T E_s         [128, 1024] PSUM (replicated over parts)
      Y^T   += V_s^T E_s           [128, 1024] PSUM
    Y^T_norm = Y^T * recip_approx(sumexp)   DVE -> SBUF fp32r
  out^T = Wu_h^T Y^T_norm   [o=128, 4096] fp32r -> DRAM

All large SBUF tensors are chunked into [128, 1024] tiles so phases
overlap at chunk granularity instead of serializing on whole-tensor deps.

Host: out = sum_c out_c^T.T + bu, reshaped to (2, 2048, 128).
"""

import sys

import numpy as np

if "/opt/trn_rl_repo" not in sys.path:
    sys.path.insert(0, "/opt/trn_rl_repo")

B, T, K, H = 2, 2048, 128, 8
BT = B * T              # 4096 tokens over both batches
NT = BT // 128          # 32 token tiles of 128
NC = BT // 1024         # 4 column chunks for the big SBUF tensors
NCORES = 8
TB = 1024               # token block (2 psum banks)
NS = T // 128           # 16 key chunks per batch
SCALE = 1.0 / np.sqrt(np.float32(K))

_compiled = None


def _build():
    import concourse.mybir as mybir
    import concourse.tile as tile
    from concourse import bacc

    f32 = mybir.dt.float32
    f32r = mybir.dt.float32r
    bf16 = mybir.dt.bfloat16
    Exp = mybir.ActivationFunctionType.Exp

    nc = bacc.Bacc(
        "TRN2",
        target_bir_lowering=False,
        debug=False,
        enable_asserts=False,
        num_devices=NCORES,
    )

    x_d = nc.dram_tensor("x", [BT, K], f32, kind="ExternalInput").ap()
    wq_d = nc.dram_tensor("wq", [K, K], f32, kind="ExternalInput").ap()
    wk_d = nc.dram_tensor("wk", [K, K], f32, kind="ExternalInput").ap()
    wv_d = nc.dram_tensor("wv", [K, K], f32, kind="ExternalInput").ap()
    wu_d = nc.dram_tensor("wu", [K, K], f32, kind="ExternalInput").ap()
    out_d = nc.dram_tensor("out", [K, BT], f32, kind="ExternalOutput").ap()

    with tile.TileContext(nc) as tc:
        from contextlib import ExitStack

        with ExitStack() as ctx:
            const = ctx.enter_context(tc.tile_pool(name="const", bufs=1))
            big = ctx.enter_context(tc.tile_pool(name="big", bufs=1))
            work = ctx.enter_context(tc.tile_pool(name="work", bufs=3))
            # PSUM budget (8 banks): s 2x[128,1024]f32 = 4, y 1x = 2, sum 1x = 2
            ps_s = ctx.enter_context(tc.tile_pool(name="ps_s", bufs=2, space="PSUM"))
            ps_y = ctx.enter_context(tc.tile_pool(name="ps_y", bufs=1, space="PSUM"))
            ps_sum = ctx.enter_context(tc.tile_pool(name="ps_sum", bufs=1, space="PSUM"))

            def chunked(tag, dtype):
                return [big.tile([128, 1024], dtype, tag=f"{tag}{c}",
                                 name=f"{tag}{c}")
                        for c in range(NC)]

            def cc(chunks, col, width):
                c, off = divmod(col, 1024)
                return chunks[c][:, off : off + width]

            # x first: everything downstream gates on it
            x_sb = []
            x_re = x_d.rearrange("(n p) k -> p n k", p=128)
            for h in range(8):
                xc = big.tile([128, 4, 128], f32, tag=f"x{h}")
                nc.sync.dma_start(xc[:], x_re[:, 4 * h : 4 * (h + 1), :])
                x_sb.append(xc)

            ones = const.tile([128, 128], bf16)
            nc.gpsimd.memset(ones[:], 1.0)

            # weights: DMA fp32, DVE cast to matmul dtypes
            wq_st = const.tile([128, 128], f32, tag="wq_st")
            wk_st = const.tile([128, 128], f32, tag="wk_st")
            wv_st = const.tile([128, 128], f32, tag="wv_st")
            wu_st = const.tile([128, 128], f32, tag="wu_st")
            nc.sync.dma_start(wv_st[:], wv_d[:])
            nc.sync.dma_start(wk_st[:], wk_d[:])
            nc.sync.dma_start(wq_st[:], wq_d[:])
            nc.sync.dma_start(wu_st[:], wu_d[:])
            wq_sb = const.tile([128, 128], bf16, tag="wq")
            wk_sb = const.tile([128, 128], bf16, tag="wk")
            wv_sb = const.tile([128, 128], bf16, tag="wv")
            wu_sb = const.tile([128, 128], bf16, tag="wu")
            nc.vector.tensor_copy(wv_sb[:], wv_st[:])
            nc.vector.tensor_copy(wk_sb[:], wk_st[:])
            nc.vector.tensor_copy(wq_sb[:], wq_st[:])
            nc.vector.tensor_copy(wu_sb[:], wu_st[:])

            # bf16 copy of x for the transposes
            x_bf = []
            for h in range(8):
                xb = big.tile([128, 4, 128], bf16, tag=f"xb{h}")
                nc.vector.tensor_copy(xb[:], x_sb[h][:])
                x_bf.append(xb)

            # X^T [k, t] bf16 via xbar DMA transposes (no PE involvement)
            xt_c = chunked("xt", bf16)
            for n in range(NT):
                nc.sync.dma_start_transpose(out=cc(xt_c, 128 * n, 128),
                                            in_=x_bf[n // 4][:, n % 4, :])

            # projections (bf16), V first and interleaved so V-transposes and
            # attention start as early as possible
            qt_c = chunked("qt", bf16)
            kt_c = chunked("kt", bf16)
            vt_c = chunked("vt", bf16)
            v_c = chunked("v", bf16)
            for blk in range(BT // 512):
                for w_sb, dst in ((wv_sb, vt_c), (wk_sb, kt_c), (wq_sb, qt_c)):
                    pp = ps_s.tile([128, 512], f32, tag="s")
                    nc.tensor.matmul(
                        pp[:],
                        w_sb[:],
                        cc(xt_c, 512 * blk, 512),
                        start=True,
                        stop=True,
                    )
                    nc.vector.tensor_copy(cc(dst, 512 * blk, 512), pp[:])
                # V chunks of this 512-block back to [s, j] layout (xbar DMA)
                for c in range(4 * blk, 4 * blk + 4):
                    nc.sync.dma_start_transpose(out=cc(v_c, 128 * c, 128),
                                                in_=cc(vt_c, 128 * c, 128))

            # attention, software-pipelined ACROSS token blocks: the S
            # matmul for key-chunk s+1 (or the next block's chunk 0) is
            # emitted ahead of the consumers of chunk s, so the PE always
            # has independent work while exp runs / psum slots recycle
            y_c = chunked("y", bf16)
            blocks = [(b, tb) for b in range(B) for tb in range(T // TB)]

            def s_matmul(blk_i, s):
                b, tb = blocks[blk_i]
                scol = b * T + s * 128
                tcol = b * T + tb * TB
                ps = ps_s.tile([128, TB], f32, tag="s", name=f"ps_{blk_i}_{s}")
                for g in range(TB // 512):
                    nc.tensor.matmul(
                        ps[:, 512 * g : 512 * (g + 1)],
                        cc(kt_c, scol, 128),
                        cc(qt_c, tcol + 512 * g, 512),
                        start=True,
                        stop=True,
                    )
                return ps

            pending = s_matmul(0, 0)
            for blk_i, (b, tb) in enumerate(blocks):
                tcol = b * T + tb * TB
                py = ps_y.tile([128, TB], f32, tag="y")
                psumt = ps_sum.tile([128, TB], f32, tag="sum")
                for s in range(NS):
                    ps = pending
                    if s + 1 < NS:
                        pending = s_matmul(blk_i, s + 1)
                    elif blk_i + 1 < len(blocks):
                        pending = s_matmul(blk_i + 1, 0)
                    scol = b * T + s * 128
                    e_sb = work.tile([128, TB], bf16, tag="e")
                    nc.scalar.activation(e_sb[:], ps[:], Exp, scale=float(SCALE))
                    for g in range(TB // 512):
                        sl = slice(512 * g, 512 * (g + 1))
                        nc.tensor.matmul(
                            psumt[:, sl],
                            ones[:],
                            e_sb[:, sl],
                            start=(s == 0),
                            stop=(s == NS - 1),
                            skip_group_check=True,
                        )
                        nc.tensor.matmul(
                            py[:, sl],
                            cc(v_c, scol, 128),
                            e_sb[:, sl],
                            start=(s == 0),
                            stop=(s == NS - 1),
                            skip_group_check=True,
                        )
                # sumexp is in [2e2, 2e4] — safely inside the approx
                # reciprocal's domain; ~18 bits is plenty for softmax
                # normalization (the exact InstReciprocal costs 6.5us)
                r_sb = work.tile([128, TB], f32, tag="r")
                nc.vector.reciprocal_approx_fast(r_sb[:], psumt[:])
                nc.vector.tensor_mul(cc(y_c, tcol, TB), py[:], r_sb[:])

                # unify this block: out^T = Wu_h^T @ Y^T (bf16)
                out_sb = big.tile([128, TB], f32, tag=f"out{tcol // TB}",
                                  name=f"out_sb{tcol // TB}")
                for g in range(TB // 512):
                    po = ps_s.tile([128, 512], f32, tag="s")
                    nc.tensor.matmul(
                        po[:],
                        wu_sb[:],
                        cc(y_c, tcol + 512 * g, 512),
                        start=True,
                        stop=True,
                    )
                    nc.vector.tensor_copy(out_sb[:, 512 * g : 512 * (g + 1)],
                                          po[:])
                nc.sync.dma_start(out_d[:, tcol : tcol + TB], out_sb[:])

    nc.compile()
    return nc


def _get_nc():
    global _compiled
    if _compiled is None:
        _compiled = _build()
    return _compiled


def kernel(x, Wq, Wk, Wv, Wu, bu, **_run_kwargs):
    from concourse.bass_utils import run_bass_kernel_spmd

    nc = _get_nc()

    x = np.ascontiguousarray(np.asarray(x, dtype=np.float32).reshape(BT, K))
    Wq = np.asarray(Wq, dtype=np.float32)
    Wk = np.asarray(Wk, dtype=np.float32)
    Wv = np.asarray(Wv, dtype=np.float32)
    Wu = np.asarray(Wu, dtype=np.float32)
    bu = np.asarray(bu, dtype=np.float32)

    in_maps = []
    for c in range(NCORES):
        sl = slice(c * K, (c + 1) * K)
        in_maps.append(
            {
                "x": x,
                "wq": np.ascontiguousarray(Wq[:, sl]),
                "wk": np.ascontiguousarray(Wk[:, sl]),
                "wv": np.ascontiguousarray(Wv[:, sl]),
                "wu": np.ascontiguousarray(Wu[sl, :]),
            }
        )

    res = run_bass_kernel_spmd(nc, in_maps, list(range(NCORES)), **_run_kwargs)

    out = np.zeros((BT, K), dtype=np.float32)
    for c in range(NCORES):
        out += res.results[c]["out"].T
    out += bu[None, :]
    result = out.reshape(B, T, K)
    if _run_kwargs:
        return result, res
    return result



# revision 5
# speedup vs baseline: 1.8531x; 1.8531x over previous
"""Multi-head attention (B=2, T=2048, H=8, K=128) on 8 TRN2 NeuronCores.

Sharding: tensor-parallel over heads - core c owns head c for both batches.
Each core computes its head's (unnormalized) attention output projected
through its slice of Wu; the host divides by the shipped per-token softmax
denominators, sums the 8 partials and adds the bias.

Per-core dataflow (features on partitions, tokens on the free axis):

  host:  xt = bf16(x^T) [128 i, 4096 t]; W slices pre-cast to bf16.
  qt/kt = Wq/Wk^T  xt     [128 d, 4096]  bf16  (W stationary, xt moving)
  V     = (xt_c)^T Wv     [128 keys, 128 d] per 128-token chunk - produced
          directly in [keys, dims] layout by making the xt chunk stationary;
          cast to fp8e4 (DVE) into [128, 2, 128] DoubleRow slabs.
  per 1024-token block, per 128-key chunk c (paired 2 at a time):
      S^T_c = kt_c qt       [128, 1024] PSUM fp32        (bf16 matmul)
      E_c   = exp(S^T_c/sqrt(128)) -> fp8e4 SBUF
              even chunks + every 8th odd: ACT Exp (direct fp8 output)
              other odd chunks: DVE Schraudolph - int8(S*a+b) IS the fp8
              bit pattern of exp (a=8 log2(e)/sqrt(128), b=56-0.3)
    per pair p (chunks 2p, 2p+1), fp8 DoubleRow matmuls (0.5 cyc/row):
      sumexp += ones8^T E_pair   [128, 1024] PSUM (replicated over parts)
      Y^T    += V_pair^T E_pair  [128, 1024] PSUM
    block tail: y_bf = bf16(Y^T) (DVE); out^T_blk = Wu^T y_bf (bf16);
    DMA out^T (PSUM->DRAM direct) and sumexp row 0.

Host: out = sum_c (out_c / sumexp_c)^T + bu, reshaped to (2, 2048, 128).

fp8 error budget (simulated offline vs fp64 truth): E+V fp8 with fp8-domain
Schraudolph on 7/16 of chunks -> rel err ~1.4e-2 (gate: 2e-2); softmax
normalization absorbs most of the correlated low-precision-E error.
"""

import sys

import numpy as np

if "/opt/trn_rl_repo" not in sys.path:
    sys.path.insert(0, "/opt/trn_rl_repo")

import ml_dtypes

B, T, K, H = 2, 2048, 128, 8
BT = B * T              # 4096 tokens over both batches
TB = 1024               # token block
NBLK = BT // TB         # 4
NCH = T // 128          # 16 key chunks per batch
NCORES = 8
SCALE = 1.0 / np.sqrt(np.float32(K))
SCHR_A = float(SCALE * np.log2(np.e) * 8.0)
SCHR_B = 56.0 - 0.3

_compiled = None


def _is_dve_chunk(c):
    # DVE (Schraudolph) takes odd chunks except every 8th -> 7/16 of chunks
    return (c % 2 == 1) and (c % 8 != 7)


def _build():
    import concourse.mybir as mybir
    import concourse.tile as tile
    from concourse import bacc

    f32 = mybir.dt.float32
    bf16 = mybir.dt.bfloat16
    fp8 = mybir.dt.float8e4
    i8 = mybir.dt.int8
    Exp = mybir.ActivationFunctionType.Exp
    Copy = mybir.ActivationFunctionType.Copy
    DR = mybir.MatmulPerfMode.DoubleRow

    nc = bacc.Bacc(
        "TRN2",
        target_bir_lowering=False,
        debug=False,
        enable_asserts=False,
        num_devices=NCORES,
    )

    xt_d = nc.dram_tensor("xt", [K, BT], bf16, kind="ExternalInput").ap()
    wq_d = nc.dram_tensor("wq", [K, K], bf16, kind="ExternalInput").ap()
    wk_d = nc.dram_tensor("wk", [K, K], bf16, kind="ExternalInput").ap()
    wv_d = nc.dram_tensor("wv", [K, K], bf16, kind="ExternalInput").ap()
    wu_d = nc.dram_tensor("wu", [K, K], bf16, kind="ExternalInput").ap()
    out_d = nc.dram_tensor("out", [K, BT], f32, kind="ExternalOutput").ap()
    sums_d = nc.dram_tensor("sums", [1, NBLK * TB], f32, kind="ExternalOutput").ap()

    with tile.TileContext(nc) as tc:
        from contextlib import ExitStack

        with ExitStack() as ctx:
            const = ctx.enter_context(tc.tile_pool(name="const", bufs=1))
            big = ctx.enter_context(tc.tile_pool(name="big", bufs=1))
            e8p = ctx.enter_context(tc.tile_pool(name="e8p", bufs=4))
            work = ctx.enter_context(tc.tile_pool(name="work", bufs=2))
            # PSUM budget (8 banks): S 2x[128,1024]f32 = 4, y 1x = 2, sum 1x = 2
            ps_s = ctx.enter_context(tc.tile_pool(name="ps_s", bufs=2, space="PSUM"))
            ps_y = ctx.enter_context(tc.tile_pool(name="ps_y", bufs=1, space="PSUM"))
            ps_sum = ctx.enter_context(tc.tile_pool(name="ps_sum", bufs=1, space="PSUM"))

            # weights + xt loads
            wq_sb = const.tile([128, 128], bf16, tag="wq")
            wk_sb = const.tile([128, 128], bf16, tag="wk")
            wv_sb = const.tile([128, 128], bf16, tag="wv")
            wu_sb = const.tile([128, 128], bf16, tag="wu")
            nc.sync.dma_start(wq_sb[:], wq_d[:])
            nc.sync.dma_start(wk_sb[:], wk_d[:])
            nc.sync.dma_start(wv_sb[:], wv_d[:])
            nc.sync.dma_start(wu_sb[:], wu_d[:])

            xt_c = []
            for c in range(4):
                xc = big.tile([128, 1024], bf16, tag=f"xt{c}", name=f"xt{c}")
                nc.sync.dma_start(xc[:], xt_d[:, 1024 * c : 1024 * (c + 1)])
                xt_c.append(xc)

            ones8 = const.tile([128, 2, 128], fp8, tag="ones8")
            nc.gpsimd.memset(ones8[:], 1.0)

            def col(chunks, c, w):
                i, off = divmod(c, 1024)
                return chunks[i][:, off : off + w]

            # Q^T / K^T projections: W stationary, xt moving -> psum -> bf16
            # (cast on ACT engine: DVE is loaded with Schraudolph exp later)
            qt_c = [big.tile([128, 1024], bf16, tag=f"qt{c}", name=f"qt{c}")
                    for c in range(4)]
            kt_c = [big.tile([128, 1024], bf16, tag=f"kt{c}", name=f"kt{c}")
                    for c in range(4)]
            for w_sb, dst in ((wq_sb, qt_c), (wk_sb, kt_c)):
                for c in range(4):
                    pp = ps_s.tile([128, 1024], f32, tag="s", name="pp")
                    for g in range(2):
                        nc.tensor.matmul(
                            pp[:, 512 * g : 512 * (g + 1)],
                            w_sb[:],
                            xt_c[c][:, 512 * g : 512 * (g + 1)],
                            start=True,
                            stop=True,
                        )
                    nc.scalar.activation(dst[c][:], pp[:], Copy)

            # V directly in [keys, dims] via stationary xt chunks, cast fp8.
            # v8_c[i] covers key chunks 4i..4i+3 = DR pairs 2i, 2i+1.
            v8_c = []
            for i in range(8):
                pv = ps_s.tile([128, 512], f32, tag="s", name="pv")
                for j in range(4):
                    ch = 4 * i + j
                    nc.tensor.matmul(
                        pv[:, 128 * j : 128 * (j + 1)],
                        col(xt_c, 128 * ch, 128),
                        wv_sb[:],
                        start=True,
                        stop=True,
                    )
                v8 = big.tile([128, 4, 128], fp8, tag=f"v8_{i}", name=f"v8_{i}")
                nc.vector.tensor_copy(
                    v8[:], pv[:].rearrange("p (a b) -> p a b", a=4)
                )
                v8_c.append(v8)

            sums_sb = const.tile([1, NBLK * TB], f32, tag="sums_sb")

            # attention: 4 blocks x 16 chunks, software-pipelined 2 ahead
            chunks = [(blk, c) for blk in range(NBLK) for c in range(NCH)]

            def s_matmul(blk, c):
                b = blk // 2
                scol = b * T + c * 128
                tcol = blk * TB
                ps = ps_s.tile([128, TB], f32, tag="s", name=f"ps_{blk}_{c}")
                for g in range(2):
                    nc.tensor.matmul(
                        ps[:, 512 * g : 512 * (g + 1)],
                        col(kt_c, scol, 128),
                        col(qt_c, tcol + 512 * g, 512),
                        start=True,
                        stop=True,
                    )
                return ps

            pend = [s_matmul(*chunks[0]), s_matmul(*chunks[1])]
            e8_pair = None
            py = psumt = None
            for ci, (blk, c) in enumerate(chunks):
                if ci % NCH == 0:
                    py = ps_y.tile([128, TB], f32, tag="y", name="py")
                    psumt = ps_sum.tile([128, TB], f32, tag="sum", name="psumt")
                ps = pend.pop(0)
                if ci + 2 < len(chunks):
                    pend.append(s_matmul(*chunks[ci + 2]))
                if c % 2 == 0:
                    e8_pair = e8p.tile([128, 2, TB], fp8, tag="e8",
                                       name=f"e8_{blk}_{c // 2}")
                if _is_dve_chunk(c):
                    nc.vector.tensor_scalar(
                        e8_pair[:, c % 2, :].bitcast(i8),
                        ps[:],
                        SCHR_A,
                        SCHR_B,
                        mybir.AluOpType.mult,
                        mybir.AluOpType.add,
                    )
                else:
                    nc.scalar.activation(
                        e8_pair[:, c % 2, :], ps[:], Exp, scale=float(SCALE)
                    )
                if c % 2 == 1:
                    pair = c // 2
                    b = blk // 2
                    vp = v8_c[(b * NCH + c - 1) // 4]
                    voff = ((c - 1) % 4) // 2 * 2
                    for g in range(2):
                        sl = slice(512 * g, 512 * (g + 1))
                        nc.tensor.matmul(
                            psumt[:, sl],
                            ones8[:],
                            e8_pair[:, :, sl],
                            start=(pair == 0),
                            stop=(pair == NCH // 2 - 1),
                            perf_mode=DR,
                            skip_group_check=True,
                        )
                        nc.tensor.matmul(
                            py[:, sl],
                            vp[:, voff : voff + 2, :],
                            e8_pair[:, :, sl],
                            start=(pair == 0),
                            stop=(pair == NCH // 2 - 1),
                            perf_mode=DR,
                            skip_group_check=True,
                        )
                if ci % NCH == NCH - 1:
                    tcol = blk * TB
                    y_sb = work.tile([128, TB], bf16, tag="ybf", name="y_sb")
                    nc.vector.tensor_copy(y_sb[:], py[:])
                    nc.scalar.activation(
                        sums_sb[0:1, blk * TB : (blk + 1) * TB],
                        psumt[0:1, :], Copy)
                    po = ps_y.tile([128, TB], f32, tag="y", name=f"po{blk}")
                    for g in range(2):
                        sl = slice(512 * g, 512 * (g + 1))
                        nc.tensor.matmul(
                            po[:, sl], wu_sb[:], y_sb[:, sl],
                            start=True, stop=True,
                        )
                    out_sb = work.tile([128, TB], f32, tag=f"out{blk}",
                                       name=f"out_sb{blk}")
                    nc.vector.tensor_copy(out_sb[:], po[:])
                    nc.sync.dma_start(out_d[:, tcol : tcol + TB], out_sb[:])
            nc.sync.dma_start(sums_d[:], sums_sb[:])

    nc.compile()
    return nc


def _get_nc():
    global _compiled
    if _compiled is None:
        _compiled = _build()
    return _compiled


def kernel(x, Wq, Wk, Wv, Wu, bu, **_run_kwargs):
    from concourse.bass_utils import run_bass_kernel_spmd

    nc = _get_nc()

    bf = ml_dtypes.bfloat16
    x = np.asarray(x, dtype=np.float32).reshape(BT, K)
    xt = np.ascontiguousarray(x.T.astype(bf))
    Wq = np.asarray(Wq, dtype=np.float32)
    Wk = np.asarray(Wk, dtype=np.float32)
    Wv = np.asarray(Wv, dtype=np.float32)
    Wu = np.asarray(Wu, dtype=np.float32)
    bu = np.asarray(bu, dtype=np.float32)

    in_maps = []
    for c in range(NCORES):
        sl = slice(c * K, (c + 1) * K)
        in_maps.append(
            {
                "xt": xt,
                "wq": np.ascontiguousarray(Wq[:, sl].astype(bf)),
                "wk": np.ascontiguousarray(Wk[:, sl].astype(bf)),
                "wv": np.ascontiguousarray(Wv[:, sl].astype(bf)),
                "wu": np.ascontiguousarray(Wu[sl, :].astype(bf)),
            }
        )

    res = run_bass_kernel_spmd(nc, in_maps, list(range(NCORES)), **_run_kwargs)

    out = np.zeros((BT, K), dtype=np.float64)
    for c in range(NCORES):
        o = np.asarray(res.results[c]["out"], dtype=np.float64)   # [128, 4096]
        s = np.asarray(res.results[c]["sums"], dtype=np.float64).reshape(BT)
        out += (o / s[None, :]).T
    out += bu[None, :].astype(np.float64)
    result = out.astype(np.float32).reshape(B, T, K)
    if _run_kwargs:
        return result, res
    return result


# revision 7
# speedup vs baseline: 2.0170x; 1.0884x over previous
"""Multi-head attention (B=2, T=2048, H=8, K=128) on 8 TRN2 NeuronCores.

Sharding: tensor-parallel over heads - core c owns head c for both batches.
Each core computes its head's (unnormalized) attention output projected
through its slice of Wu; the host divides by the shipped per-token softmax
denominators, sums the 8 partials and adds the bias.

Per-core dataflow (features on partitions, tokens on the free axis):

  host:  xt = bf16(x^T) [128 i, 4096 t]; W slices pre-cast to bf16.
  qt/kt = Wq/Wk^T  xt     [128 d, 4096]  bf16  (W stationary, xt moving)
  V     = (xt_c)^T Wv     [128 keys, 128 d] per 128-token chunk - produced
          directly in [keys, dims] layout by making the xt chunk stationary;
          cast to fp8e4 (DVE) into DoubleRow slabs.
  per 1024-token block, per 128-key chunk c (3-deep software pipeline):
      S^T_c = kt_c qt       [128, 1024] PSUM fp32        (bf16 matmul)
      E_c   = exp(S^T_c/sqrt(128)) -> fp8e4 SBUF
              even chunks + every 8th odd: ACT Exp (direct fp8 output)
              other odd chunks: DVE Schraudolph - int8(S*a+b) IS the fp8
              bit pattern of exp (a=8 log2(e)/sqrt(128), b=56-0.3)
    per pair p (chunks 2p, 2p+1), fp8 DoubleRow matmul (0.5 cyc/row):
      Y^T += V_pair^T E_pair     [128, 1024] PSUM
    block tail (deferred into the next block's first chunks so no engine
    queue stalls at the boundary):
      sumexp = ones8^T E_pair, burst over all 8 pairs   (fp8 DR)
      y_bf = bf16(Y^T) (DVE); out^T_blk = Wu^T y_bf (bf16);
      out^T -> SBUF (DVE) -> DRAM; sumexp row 0 -> SBUF (ACT) -> DRAM.

Host: out = sum_c (out_c / sumexp_c)^T + bu, reshaped to (2, 2048, 128).

fp8 error budget (simulated offline vs fp64 truth): E+V fp8 with fp8-domain
Schraudolph on 7/16 of chunks -> rel err ~1.4e-2 (gate: 2e-2); softmax
normalization absorbs most of the correlated low-precision-E error.

PSUM (8 banks): ps pool 3x[128,1024]f32 (S tiles / sum burst / out proj,
rotating) = 6 banks, ps_y 1x[128,1024] (Y accumulator) = 2 banks.
"""

import sys

import numpy as np

if "/opt/trn_rl_repo" not in sys.path:
    sys.path.insert(0, "/opt/trn_rl_repo")

import ml_dtypes

B, T, K, H = 2, 2048, 128, 8
BT = B * T              # 4096 tokens over both batches
TB = 1024               # token block
NBLK = BT // TB         # 4
NCH = T // 128          # 16 key chunks per batch
NCORES = 8
SCALE = 1.0 / np.sqrt(np.float32(K))
SCHR_A = float(SCALE * np.log2(np.e) * 8.0)
SCHR_B = 56.0 - 0.3

_compiled = None


def _is_dve_chunk(c):
    # DVE (Schraudolph) takes odd chunks except every 8th -> 7/16 of chunks
    return (c % 2 == 1) and (c % 8 != 7)


def _build():
    import concourse.mybir as mybir
    import concourse.tile as tile
    from concourse import bacc

    f32 = mybir.dt.float32
    bf16 = mybir.dt.bfloat16
    fp8 = mybir.dt.float8e4
    i8 = mybir.dt.int8
    Exp = mybir.ActivationFunctionType.Exp
    Copy = mybir.ActivationFunctionType.Copy
    DR = mybir.MatmulPerfMode.DoubleRow

    nc = bacc.Bacc(
        "TRN2",
        target_bir_lowering=False,
        debug=False,
        enable_asserts=False,
        num_devices=NCORES,
    )

    xt_d = nc.dram_tensor("xt", [K, BT], bf16, kind="ExternalInput").ap()
    wq_d = nc.dram_tensor("wq", [K, K], bf16, kind="ExternalInput").ap()
    wk_d = nc.dram_tensor("wk", [K, K], bf16, kind="ExternalInput").ap()
    wv_d = nc.dram_tensor("wv", [K, K], bf16, kind="ExternalInput").ap()
    wu_d = nc.dram_tensor("wu", [K, K], bf16, kind="ExternalInput").ap()
    out_d = nc.dram_tensor("out", [K, BT], f32, kind="ExternalOutput").ap()
    sums_d = nc.dram_tensor("sums", [1, BT], f32, kind="ExternalOutput").ap()

    with tile.TileContext(nc) as tc:
        from contextlib import ExitStack

        with ExitStack() as ctx:
            const = ctx.enter_context(tc.tile_pool(name="const", bufs=1))
            big = ctx.enter_context(tc.tile_pool(name="big", bufs=1))
            e8p = ctx.enter_context(tc.tile_pool(name="e8p", bufs=11))
            work = ctx.enter_context(tc.tile_pool(name="work", bufs=2))
            ps = ctx.enter_context(tc.tile_pool(name="ps", bufs=3, space="PSUM"))
            ps_y = ctx.enter_context(tc.tile_pool(name="ps_y", bufs=1, space="PSUM"))

            # inputs: xt chunk 0 + weights first (gate the projections), xt
            # rest on the idle gpsimd queue so the DMAs issue in parallel
            xt_c = [big.tile([128, 1024], bf16, tag=f"xt{c}", name=f"xt{c}")
                    for c in range(4)]
            nc.sync.dma_start(xt_c[0][:], xt_d[:, 0:1024])
            wq_sb = const.tile([128, 128], bf16, tag="wq")
            wk_sb = const.tile([128, 128], bf16, tag="wk")
            wv_sb = const.tile([128, 128], bf16, tag="wv")
            wu_sb = const.tile([128, 128], bf16, tag="wu")
            nc.scalar.dma_start(wq_sb[:], wq_d[:])
            nc.scalar.dma_start(wk_sb[:], wk_d[:])
            nc.scalar.dma_start(wv_sb[:], wv_d[:])
            nc.scalar.dma_start(wu_sb[:], wu_d[:])
            for c in range(1, 4):
                nc.gpsimd.dma_start(xt_c[c][:], xt_d[:, 1024 * c : 1024 * (c + 1)])

            ones8 = const.tile([128, 2, 128], fp8, tag="ones8")
            nc.gpsimd.memset(ones8[:], 1.0)
            sums_sb = const.tile([1, BT], f32, tag="sums_sb")

            def col(chunks, c, w):
                i, off = divmod(c, 1024)
                return chunks[i][:, off : off + w]

            # Q^T / K^T projections: W stationary, xt moving -> psum -> bf16
            # (cast on ACT: DVE is loaded with Schraudolph exp later)
            qt_c = [big.tile([128, 1024], bf16, tag=f"qt{c}", name=f"qt{c}")
                    for c in range(4)]
            kt_c = [big.tile([128, 1024], bf16, tag=f"kt{c}", name=f"kt{c}")
                    for c in range(4)]
            for w_sb, dst in ((wq_sb, qt_c), (wk_sb, kt_c)):
                for c in range(4):
                    pp = ps.tile([128, 1024], f32, tag="s", name="pp")
                    for g in range(2):
                        nc.tensor.matmul(
                            pp[:, 512 * g : 512 * (g + 1)],
                            w_sb[:],
                            xt_c[c][:, 512 * g : 512 * (g + 1)],
                            start=True,
                            stop=True,
                        )
                    nc.scalar.activation(dst[c][:], pp[:], Copy)

            # V directly in [keys, dims] via stationary xt chunks, cast fp8.
            # v8_c[i] covers key chunks 4i..4i+3 = DR pairs 2i, 2i+1.
            v8_c = []
            for i in range(8):
                pv = ps.tile([128, 512], f32, tag="s", name="pv")
                for j in range(4):
                    ch = 4 * i + j
                    nc.tensor.matmul(
                        pv[:, 128 * j : 128 * (j + 1)],
                        col(xt_c, 128 * ch, 128),
                        wv_sb[:],
                        start=True,
                        stop=True,
                    )
                v8 = big.tile([128, 4, 128], fp8, tag=f"v8_{i}", name=f"v8_{i}")
                nc.vector.tensor_copy(
                    v8[:], pv[:].rearrange("p (a b) -> p a b", a=4)
                )
                v8_c.append(v8)

            # attention: 4 blocks x 16 chunks, 3-deep software pipeline with
            # block tails deferred into the next block's first chunks
            chunks = [(blk, c) for blk in range(NBLK) for c in range(NCH)]
            DEPTH = 3

            def s_matmul(blk, c):
                b = blk // 2
                scol = b * T + c * 128
                tcol = blk * TB
                sp = ps.tile([128, TB], f32, tag="s", name=f"ps_{blk}_{c}")
                for g in range(2):
                    nc.tensor.matmul(
                        sp[:, 512 * g : 512 * (g + 1)],
                        col(kt_c, scol, 128),
                        col(qt_c, tcol + 512 * g, 512),
                        start=True,
                        stop=True,
                    )
                return sp

            def tail(blk, step, st):
                # deferred tail of block `blk`; step advances with the next
                # block's chunks so no engine queue blocks at the boundary
                if step == 0:
                    st["ybf"] = work.tile([128, TB], bf16, tag="ybf",
                                          name=f"ybf{blk}")
                    nc.vector.tensor_copy(st["ybf"][:], st["py"][:])
                    psumt = ps.tile([128, TB], f32, tag="s", name=f"sum{blk}")
                    for pair in range(8):
                        for g in range(2):
                            sl = slice(512 * g, 512 * (g + 1))
                            nc.tensor.matmul(
                                psumt[:, sl],
                                ones8[:],
                                st["e8"][pair][:, :, sl],
                                start=(pair == 0),
                                stop=(pair == 7),
                                perf_mode=DR,
                            )
                    st["psumt"] = psumt
                elif step == 1:
                    nc.scalar.activation(
                        sums_sb[0:1, blk * TB : (blk + 1) * TB],
                        st["psumt"][0:1, :], Copy)
                    po = ps.tile([128, TB], f32, tag="s", name=f"po{blk}")
                    for g in range(2):
                        sl = slice(512 * g, 512 * (g + 1))
                        nc.tensor.matmul(
                            po[:, sl], wu_sb[:], st["ybf"][:, sl],
                            start=True, stop=True,
                        )
                    st["po"] = po
                elif step == 2:
                    out_sb = work.tile([128, TB], f32, tag="outsb",
                                       name=f"osb{blk}")
                    nc.vector.tensor_copy(out_sb[:], st["po"][:])
                    nc.sync.dma_start(
                        out_d[:, blk * TB : (blk + 1) * TB], out_sb[:])

            pend = [s_matmul(*chunks[i]) for i in range(DEPTH)]
            prev_tail = None
            st = None
            for ci, (blk, c) in enumerate(chunks):
                if c == 0:
                    st = {"py": ps_y.tile([128, TB], f32, tag="y",
                                          name=f"py{blk}"),
                          "e8": []}
                sp = pend.pop(0)
                if ci + DEPTH < len(chunks):
                    pend.append(s_matmul(*chunks[ci + DEPTH]))
                if c % 2 == 0:
                    st["e8"].append(e8p.tile([128, 2, TB], fp8, tag="e8",
                                             name=f"e8_{blk}_{c // 2}"))
                e8_pair = st["e8"][c // 2]
                if _is_dve_chunk(c):
                    nc.vector.tensor_scalar(
                        e8_pair[:, c % 2, :].bitcast(i8),
                        sp[:],
                        SCHR_A,
                        SCHR_B,
                        mybir.AluOpType.mult,
                        mybir.AluOpType.add,
                    )
                else:
                    nc.scalar.activation(
                        e8_pair[:, c % 2, :], sp[:], Exp, scale=float(SCALE)
                    )
                if c % 2 == 1:
                    pair = c // 2
                    b = blk // 2
                    vp = v8_c[(b * NCH + c - 1) // 4]
                    voff = ((c - 1) % 4) // 2 * 2
                    for g in range(2):
                        sl = slice(512 * g, 512 * (g + 1))
                        nc.tensor.matmul(
                            st["py"][:, sl],
                            vp[:, voff : voff + 2, :],
                            e8_pair[:, :, sl],
                            start=(pair == 0),
                            stop=(pair == 7),
                            perf_mode=DR,
                            skip_group_check=True,
                        )
                if prev_tail is not None and c in (0, 1, 2):
                    tail(prev_tail[0], c, prev_tail[1])
                    if c == 2:
                        prev_tail = None
                if c == NCH - 1:
                    prev_tail = (blk, st)
            for step in range(3):
                tail(prev_tail[0], step, prev_tail[1])
            nc.sync.dma_start(sums_d[:], sums_sb[:])

    nc.compile()
    return nc


def _get_nc():
    global _compiled
    if _compiled is None:
        _compiled = _build()
    return _compiled


def kernel(x, Wq, Wk, Wv, Wu, bu, **_run_kwargs):
    from concourse.bass_utils import run_bass_kernel_spmd

    nc = _get_nc()

    bf = ml_dtypes.bfloat16
    x = np.asarray(x, dtype=np.float32).reshape(BT, K)
    xt = np.ascontiguousarray(x.T.astype(bf))
    Wq = np.asarray(Wq, dtype=np.float32)
    Wk = np.asarray(Wk, dtype=np.float32)
    Wv = np.asarray(Wv, dtype=np.float32)
    Wu = np.asarray(Wu, dtype=np.float32)
    bu = np.asarray(bu, dtype=np.float32)

    in_maps = []
    for c in range(NCORES):
        sl = slice(c * K, (c + 1) * K)
        in_maps.append(
            {
                "xt": xt,
                "wq": np.ascontiguousarray(Wq[:, sl].astype(bf)),
                "wk": np.ascontiguousarray(Wk[:, sl].astype(bf)),
                "wv": np.ascontiguousarray(Wv[:, sl].astype(bf)),
                "wu": np.ascontiguousarray(Wu[sl, :].astype(bf)),
            }
        )

    res = run_bass_kernel_spmd(nc, in_maps, list(range(NCORES)), **_run_kwargs)

    out = np.zeros((BT, K), dtype=np.float64)
    for c in range(NCORES):
        o = np.asarray(res.results[c]["out"], dtype=np.float64)   # [128, 4096]
        s = np.asarray(res.results[c]["sums"], dtype=np.float64).reshape(BT)
        out += (o / s[None, :]).T
    out += bu[None, :].astype(np.float64)
    result = out.astype(np.float32).reshape(B, T, K)
    if _run_kwargs:
        return result, res
    return result
